# revision 34
# baseline (speedup 1.0000x reference)
"""Trainium2 Bass kernel for causal degree-2 polynomial attention.

The reference module is chunked linear attention with kernel weight
(q.k)^2, which is mathematically exact causal polynomial attention:

    out_q = sum_{k<=q} (Q_q.K_k)^2 V_k / (EPS + sum_{k<=q} (Q_q.K_k)^2)

Sharding: 16 (batch, head) pairs across 8 cores -> 2 pairs/core, fully
data-parallel (matches the chunk-local-cumsum hint; no collectives).

Host-side prep (part of the shard/layout step): Q^T / K^T / [V|1] are
laid out in bf16 exactly as the device consumes them, so the device
does no transposes and no casts:
  - qT2 [128, 2048]: Q^T duplicated on both partition halves (feeds the
    two concurrent K=64 PE tiles)
  - kT2 [128, 1024]: col group a holds K^T of key block 2a (top half)
    and 2a+1 (bottom half)
  - v1  [128, 16*65]: [V_block | ones] per key block (ones col computes
    the normalizer Z on the same matmul)

Per-core device algorithm (two pairs interleaved to fill stalls):
  - D'[k, q] = K Q^T per (512-query block i, 256-key dpair a) as two
    concurrent row-tiled K=64 bf16 matmuls, narrowed to causal cols.
  - exit PSUM -> bf16 SBUF with squaring, greedily balanced between
    ACT (direct square) and DVE copy + Pool/DVE bf16 multiply; the
    diagonal 128-col windows get an in-place Pool multiply with a
    host-provided triangular mask.
  - accumulate [V|1]^T C' into PSUM [65, 512] (bf16), causally
    narrowed; copy to SBUF bf16 and store raw [Y^T; Z] blocks.
Host epilogue: out = (Y^T / Z)^T per query block (EPS dropped:
Z >= (q.q)^2 >> 1e5*EPS).
"""

import os
import sys

for _p in ("/root/.axon_site", "/root/.axon_site/_ro/trn_rl_repo",
           "/root/.axon_site/_ro/pypackages", "/opt/trn_rl_repo", "/opt/pypackages"):
    if os.path.isdir(_p) and _p not in sys.path:
        sys.path.append(_p)

import ml_dtypes
import numpy as np

import concourse.bacc as bacc
import concourse.mybir as mybir
import concourse.tile as tile
from concourse.bass_utils import run_bass_kernel_spmd

F32 = mybir.dt.float32
BF16 = mybir.dt.bfloat16
NP_BF16 = np.dtype(ml_dtypes.bfloat16)

N_CORES = 8
T = 2048          # tokens
D = 64            # head dim
PAIRS = 2         # (b, h) pairs per core
NKB = T // 128    # 16 key blocks of 128
QB = 512          # query block width
NQB = T // QB     # 4 query blocks

_CACHE = {}


def _interleave(lists):
    out = []
    n = max(len(l) for l in lists)
    for j in range(n):
        for l in lists:
            if j < len(l):
                out.append(l[j])
    return out


def build_nc():
    nc = bacc.Bacc("TRN2", target_bir_lowering=False, debug=False)

    ins = {}
    outs = {}
    for p in range(PAIRS):
        ins[f"qT2_{p}"] = nc.dram_tensor(f"qT2_{p}", [128, T], BF16, kind="ExternalInput").ap()
        ins[f"kT2_{p}"] = nc.dram_tensor(f"kT2_{p}", [128, T // 2], BF16, kind="ExternalInput").ap()
        ins[f"v1_{p}"] = nc.dram_tensor(f"v1_{p}", [128, NKB * 65], BF16, kind="ExternalInput").ap()
        outs[p] = nc.dram_tensor(f"o{p}", [NQB * 65, QB], BF16, kind="ExternalOutput").ap()
    trimask = nc.dram_tensor("trimask", [128, 128], BF16, kind="ExternalInput").ap()

    # estimated busy-ns per engine, for greedy exit routing
    busy = {"A": 0.0, "V": 0.0, "P": 0.0}

    def add(deltas):
        for k, v in deltas.items():
            busy[k] += v

    def peak(deltas):
        return max(busy[k] + deltas.get(k, 0.0) for k in busy)

    with tile.TileContext(nc) as tc:
        with (
            tc.tile_pool(name="const", bufs=1) as cpool,
            tc.tile_pool(name="persist", bufs=1) as perpool,
            tc.tile_pool(name="cprime", bufs=6) as cppool,
            tc.tile_pool(name="dstage", bufs=3) as dpool,
            tc.tile_pool(name="small", bufs=4) as smpool,
            tc.tile_pool(name="psd", bufs=3, space="PSUM") as psd,
            tc.tile_pool(name="psyz", bufs=2, space="PSUM") as psyz,
        ):
            trimask_sb = cpool.tile([128, 128], BF16, name="trimask_sb")
            nc.sync.dma_start(trimask_sb[:], trimask[:])

            qT2 = []
            kT2 = []
            v1 = []
            for p in range(PAIRS):
                qT2.append(perpool.tile([128, T], BF16, name=f"qT2_{p}"))
                kT2.append(perpool.tile([128, T // 2], BF16, name=f"kT2_{p}"))
                v1.append(perpool.tile([128, NKB * 65], BF16, name=f"v1_{p}"))

            # ---- input loads, ordered by first use; the two halves of a
            # pair's first working set ride different queues so the first
            # D' can issue ~0.7us earlier ----
            for p in range(PAIRS):
                dmae = nc.sync if p == 0 else nc.scalar
                dmae2 = nc.scalar if p == 0 else nc.sync
                dmae.dma_start(qT2[p][:, 3 * QB:4 * QB], ins[f"qT2_{p}"][:, 3 * QB:4 * QB])
                dmae2.dma_start(kT2[p][:, 0:QB], ins[f"kT2_{p}"][:, 0:QB])
            for p in range(PAIRS):
                dmae = nc.sync if p == 0 else nc.scalar
                dmae.dma_start(kT2[p][:, QB:2 * QB], ins[f"kT2_{p}"][:, QB:2 * QB])
                dmae.dma_start(v1[p][:, 0:8 * 65], ins[f"v1_{p}"][:, 0:8 * 65])
                dmae.dma_start(v1[p][:, 8 * 65:NKB * 65], ins[f"v1_{p}"][:, 8 * 65:NKB * 65])
                dmae.dma_start(qT2[p][:, 2 * QB:3 * QB], ins[f"qT2_{p}"][:, 2 * QB:3 * QB])
                dmae.dma_start(qT2[p][:, QB:2 * QB], ins[f"qT2_{p}"][:, QB:2 * QB])
                dmae.dma_start(qT2[p][:, 0:QB], ins[f"qT2_{p}"][:, 0:QB])

            tails = {}

            def exit_square(dst, src, cols):
                """PSUM->SBUF squaring exit, greedily balanced with
                measured per-op costs (ns). ACT squares directly; DVE
                copies (cast) then bf16-muls at 2X (a 2-PSUM-operand
                tensor_tensor is rejected by the BIR verifier)."""
                optA = {"A": cols * 0.833 + 396}
                optV = {"V": cols * 1.562 + 340}
                best = min((optA, optV), key=peak)
                add(best)
                if best is optA:
                    nc.scalar.square(dst, src)
                else:
                    dstg = dpool.tile([128, 2 * QB], BF16, name="dstg", tag="dstg")
                    nc.vector.tensor_copy(dstg[:, 0:cols], src)
                    nc.vector.tensor_mul(dst, dstg[:, 0:cols], dstg[:, 0:cols])

            def exit_copy(dst, src, cols):
                optA = {"A": cols * 0.833 + 396}
                optV = {"V": cols * 1.042 + 170}
                best = min((optA, optV), key=peak)
                add(best)
                if best is optA:
                    nc.scalar.copy(dst, src)
                else:
                    nc.vector.tensor_copy(dst, src)

            def mask_window(c2, w):
                """in-place triangular mask on a 128-col diagonal window;
                DVE 2X bf16 unless Pool is the lighter engine."""
                optV = {"V": 128 * 0.52 + 170}
                optP = {"P": 128 * 3.12 + 250}
                best = min((optV, optP), key=peak)
                add(best)
                eng = nc.vector if best is optV else nc.gpsimd
                eng.tensor_mul(c2[:, w:w + 128], c2[:, w:w + 128], trimask_sb[:])

            def emit_dpair(p, i, a):
                """D'[k, q] for key blocks (2a, 2a+1) vs query block i,
                narrowed to causal cols; returns (c2 tile, w0E, w0O)."""
                kcols = slice(a * 128, (a + 1) * 128)
                psAB = psd.tile([128, 2 * QB], F32, name="psAB", tag="psd")
                w0E = max(0, 128 * (2 * a) - QB * i)
                w0O = max(0, 128 * (2 * a + 1) - QB * i)
                nc.tensor.matmul(
                    psAB[:, w0E:QB], kT2[p][0:64, kcols],
                    qT2[p][0:64, i * QB + w0E:(i + 1) * QB],
                    start=True, stop=True, tile_position=(0, 0),
                    skip_group_check=True,
                )
                nc.tensor.matmul(
                    psAB[:, QB + w0O:2 * QB], kT2[p][64:128, kcols],
                    qT2[p][64:128, i * QB + w0O:(i + 1) * QB],
                    start=True, stop=True, tile_position=(64, 0),
                    skip_group_check=True,
                )
                c2 = cppool.tile([128, 2 * QB], BF16, name="c2", tag="cp")
                if w0E == 0 and w0O == 0:
                    # fused 2-bank exit amortizes the fixed per-op cost
                    exit_square(c2[:], psAB[:], 2 * QB)
                else:
                    exit_square(c2[:, w0E:QB], psAB[:, w0E:QB], QB - w0E)
                    exit_square(c2[:, QB + w0O:2 * QB], psAB[:, QB + w0O:2 * QB],
                                QB - w0O)
                for par, w0 in ((0, w0E), (1, w0O)):
                    r = 2 * a + par - (QB // 128) * i
                    if 0 <= r < QB // 128:
                        mask_window(c2, par * QB + w0)
                return c2, w0E, w0O

            def emit_cv(p, i, a, cur):
                c2, w0E, w0O = cur
                yzp = tails[(p, i)]
                vE = v1[p][:, (2 * a) * 65:(2 * a + 1) * 65]
                vO = v1[p][:, (2 * a + 1) * 65:(2 * a + 2) * 65]
                nc.tensor.matmul(
                    yzp[:, w0E:QB], vE, c2[:, w0E:QB],
                    start=(a == 0), stop=False, skip_group_check=True,
                )
                nc.tensor.matmul(
                    yzp[:, w0O:QB], vO, c2[:, QB + w0O:2 * QB],
                    start=False, stop=(a == 2 * i + 1), skip_group_check=True,
                )

            def emit_tail(p, i):
                yzs = smpool.tile([65, QB], BF16, name="yzs", tag="yzs")
                exit_copy(yzs[:], tails[(p, i)][:], QB)
                dmae = nc.sync if p == 0 else nc.scalar
                dmae.dma_start(outs[p][i * 65:(i + 1) * 65, :], yzs[:])

            AHEAD = 2  # super-steps between a dpair's D' and its CV
            seq = [(i, a) for i in (3, 2, 1, 0) for a in range(2 * i + 2)]
            dp = {}

            def emit_d(p, j):
                dp[(p, j)] = emit_dpair(p, *seq[j])

            def emit_c(p, j):
                i, a = seq[j]
                cur = dp.pop((p, j))
                if a == 0:
                    tails[(p, i)] = psyz.tile(
                        [65, QB], F32, name=f"yzp_{p}_{i}", tag="yzp"
                    )
                emit_cv(p, i, a, cur)
                if a == 2 * i + 1:
                    emit_tail(p, i)

            # D' units run AHEAD super-steps before their CV so the PE
            # always has an independent matmul between dependent ones and
            # the exit engines get a full pipeline of slack
            for j in range(len(seq) + AHEAD):
                for p in range(PAIRS):
                    if j >= AHEAD:
                        emit_c(p, j - AHEAD)
                for p in range(PAIRS):
                    if j < len(seq):
                        emit_d(p, j)

    nc.compile()
    return nc


def _shard_inputs(Q, K, V):
    """Per-core in_maps; core c gets global (b,h) pairs 2c and 2c+1.

    Host-side layout prep: bf16 cast + transpose into the exact SBUF
    layouts the device consumes (no device-side transposes/casts).
    """
    Q = np.asarray(Q, dtype=np.float32)
    K = np.asarray(K, dtype=np.float32)
    V = np.asarray(V, dtype=np.float32)
    b, t, h, d = Q.shape
    trimask = np.triu(np.ones((128, 128), dtype=np.float32)).astype(NP_BF16)
    in_maps = []
    for c in range(N_CORES):
        m = {"trimask": trimask}
        for p in range(PAIRS):
            g = PAIRS * c + p
            bb, hh = divmod(g, h)
            qT = np.ascontiguousarray(Q[bb, :, hh, :].T).astype(NP_BF16)  # [64, 2048]
            kT = np.ascontiguousarray(K[bb, :, hh, :].T).astype(NP_BF16)
            vp = V[bb, :, hh, :].astype(NP_BF16)                          # [2048, 64]
            m[f"qT2_{p}"] = np.concatenate([qT, qT], axis=0)              # [128, 2048]
            kT2 = np.empty((128, T // 2), dtype=NP_BF16)
            kTb = kT.reshape(64, NKB, 128)
            kT2[0:64] = kTb[:, 0::2, :].reshape(64, T // 2)
            kT2[64:128] = kTb[:, 1::2, :].reshape(64, T // 2)
            m[f"kT2_{p}"] = kT2
            v1 = np.ones((128, NKB, 65), dtype=NP_BF16)
            v1[:, :, 0:64] = vp.reshape(NKB, 128, 64).transpose(1, 0, 2)
            m[f"v1_{p}"] = v1.reshape(128, NKB * 65)
        in_maps.append(m)
    return in_maps


def kernel(Q, K, V, chunk_count, trace=False):
    Q = np.asarray(Q)
    b, t, h, d = Q.shape
    assert (b, t, h, d) == (2, T, 8, D), (b, t, h, d)
    assert T % int(chunk_count) == 0

    if "nc" not in _CACHE:
        _CACHE["nc"] = build_nc()
    nc = _CACHE["nc"]

    in_maps = _shard_inputs(Q, K, V)
    res = run_bass_kernel_spmd(nc, in_maps, core_ids=list(range(N_CORES)), trace=trace)

    out = np.empty((b, t, h, d), dtype=np.float32)
    for c in range(N_CORES):
        for p in range(PAIRS):
            g = PAIRS * c + p
            bb, hh = divmod(g, h)
            arr = np.asarray(res.results[c][f"o{p}"]).astype(np.float32)
            arr = arr.reshape(NQB, 65, QB)
            y = arr[:, :64, :]                      # [NQB, 64, QB]
            z = arr[:, 64, :]                       # [NQB, QB]
            outp = (y / z[:, None, :]).transpose(0, 2, 1).reshape(T, D)
            out[bb, :, hh, :] = outp
    if trace:
        return out, res
    return out


# revision 35
# speedup vs baseline: 1.0478x; 1.0478x over previous
"""Trainium2 Bass kernel for causal degree-2 polynomial attention.

The reference module is chunked linear attention with kernel weight
(q.k)^2, which is mathematically exact causal polynomial attention:

    out_q = sum_{k<=q} (Q_q.K_k)^2 V_k / (EPS + sum_{k<=q} (Q_q.K_k)^2)

Sharding: 16 (batch, head) pairs across 8 cores -> 2 pairs/core, fully
data-parallel (matches the chunk-local-cumsum hint; no collectives).

Host-side prep (part of the shard/layout step): Q^T / K^T / [V|1] are
laid out in bf16 exactly as the device consumes them, so the device
does no transposes and no casts:
  - qT2 [128, 2048]: Q^T duplicated on both partition halves (feeds the
    two concurrent K=64 PE tiles)
  - kT2 [128, 1024]: col group a holds K^T of key block 2a (top half)
    and 2a+1 (bottom half)
  - v1  [128, 16*65]: [V_block | ones] per key block (ones col computes
    the normalizer Z on the same matmul)

Per-core device algorithm (two pairs interleaved to fill stalls):
  - D'[k, q] = K Q^T per (512-query block i, 256-key dpair a) as two
    concurrent row-tiled K=64 bf16 matmuls, narrowed to causal cols.
  - exit PSUM -> bf16 SBUF with squaring, greedily balanced between
    ACT (direct square) and DVE copy + Pool/DVE bf16 multiply; the
    diagonal 128-col windows get an in-place Pool multiply with a
    host-provided triangular mask.
  - accumulate [V|1]^T C' into PSUM [65, 512] (bf16), causally
    narrowed; copy to SBUF bf16 and store raw [Y^T; Z] blocks.
Host epilogue: out = (Y^T / Z)^T per query block (EPS dropped:
Z >= (q.q)^2 >> 1e5*EPS).
"""

import os
import sys

for _p in ("/root/.axon_site", "/root/.axon_site/_ro/trn_rl_repo",
           "/root/.axon_site/_ro/pypackages", "/opt/trn_rl_repo", "/opt/pypackages"):
    if os.path.isdir(_p) and _p not in sys.path:
        sys.path.append(_p)

import ml_dtypes
import numpy as np

import concourse.bacc as bacc
import concourse.mybir as mybir
import concourse.tile as tile
from concourse.bass_utils import run_bass_kernel_spmd

F32 = mybir.dt.float32
BF16 = mybir.dt.bfloat16
NP_BF16 = np.dtype(ml_dtypes.bfloat16)

N_CORES = 8
T = 2048          # tokens
D = 64            # head dim
PAIRS = 2         # (b, h) pairs per core
NKB = T // 128    # 16 key blocks of 128
QB = 512          # query block width
NQB = T // QB     # 4 query blocks

_CACHE = {}


def _interleave(lists):
    out = []
    n = max(len(l) for l in lists)
    for j in range(n):
        for l in lists:
            if j < len(l):
                out.append(l[j])
    return out


def build_nc():
    nc = bacc.Bacc("TRN2", target_bir_lowering=False, debug=False)

    ins = {}
    outs = {}
    for p in range(PAIRS):
        ins[f"qT2_{p}"] = nc.dram_tensor(f"qT2_{p}", [128, T], BF16, kind="ExternalInput").ap()
        ins[f"kT2_{p}"] = nc.dram_tensor(f"kT2_{p}", [128, T // 2], BF16, kind="ExternalInput").ap()
        ins[f"v1_{p}"] = nc.dram_tensor(f"v1_{p}", [128, NKB * 65], BF16, kind="ExternalInput").ap()
        outs[p] = nc.dram_tensor(f"o{p}", [NQB * 65, QB], BF16, kind="ExternalOutput").ap()
    trimask = nc.dram_tensor("trimask", [128, 128], BF16, kind="ExternalInput").ap()

    # estimated busy-ns per engine, for greedy exit routing
    busy = {"A": 0.0, "V": 0.0, "P": 0.0}

    def add(deltas):
        for k, v in deltas.items():
            busy[k] += v

    def peak(deltas):
        return max(busy[k] + deltas.get(k, 0.0) for k in busy)

    with tile.TileContext(nc) as tc:
        with (
            tc.tile_pool(name="const", bufs=1) as cpool,
            tc.tile_pool(name="persist", bufs=1) as perpool,
            tc.tile_pool(name="cprime", bufs=6) as cppool,
            tc.tile_pool(name="dstage", bufs=3) as dpool,
            tc.tile_pool(name="small", bufs=4) as smpool,
            tc.tile_pool(name="psd", bufs=3, space="PSUM") as psd,
            tc.tile_pool(name="psyz", bufs=2, space="PSUM") as psyz,
        ):
            trimask_sb = cpool.tile([128, 128], BF16, name="trimask_sb")
            nc.sync.dma_start(trimask_sb[:], trimask[:])

            qT2 = []
            kT2 = []
            v1 = []
            for p in range(PAIRS):
                qT2.append(perpool.tile([128, T], BF16, name=f"qT2_{p}"))
                kT2.append(perpool.tile([128, T // 2], BF16, name=f"kT2_{p}"))
                v1.append(perpool.tile([128, NKB * 65], BF16, name=f"v1_{p}"))

            # ---- input loads, ordered by first use; the two halves of a
            # pair's first working set ride different queues so the first
            # D' can issue ~0.7us earlier ----
            for p in range(PAIRS):
                dmae = nc.sync if p == 0 else nc.scalar
                dmae2 = nc.scalar if p == 0 else nc.sync
                dmae.dma_start(qT2[p][:, 3 * QB:4 * QB], ins[f"qT2_{p}"][:, 3 * QB:4 * QB])
                dmae2.dma_start(kT2[p][:, 0:QB], ins[f"kT2_{p}"][:, 0:QB])
            for p in range(PAIRS):
                dmae = nc.sync if p == 0 else nc.scalar
                dmae.dma_start(kT2[p][:, QB:2 * QB], ins[f"kT2_{p}"][:, QB:2 * QB])
                dmae.dma_start(v1[p][:, 0:8 * 65], ins[f"v1_{p}"][:, 0:8 * 65])
                dmae.dma_start(v1[p][:, 8 * 65:NKB * 65], ins[f"v1_{p}"][:, 8 * 65:NKB * 65])
                dmae.dma_start(qT2[p][:, 2 * QB:3 * QB], ins[f"qT2_{p}"][:, 2 * QB:3 * QB])
                dmae.dma_start(qT2[p][:, QB:2 * QB], ins[f"qT2_{p}"][:, QB:2 * QB])
                dmae.dma_start(qT2[p][:, 0:QB], ins[f"qT2_{p}"][:, 0:QB])

            tails = {}

            def exit_square(dst, src, cols):
                """PSUM->SBUF squaring exit, greedily balanced with
                measured per-op costs (ns). ACT squares directly; DVE
                copies (cast) then bf16-muls at 2X (a 2-PSUM-operand
                tensor_tensor is rejected by the BIR verifier)."""
                optA = {"A": cols * 0.833 + 396}
                optV = {"V": cols * 1.562 + 340}
                best = min((optA, optV), key=peak)
                add(best)
                if best is optA:
                    nc.scalar.square(dst, src)
                else:
                    dstg = dpool.tile([128, 2 * QB], BF16, name="dstg", tag="dstg")
                    nc.vector.tensor_copy(dstg[:, 0:cols], src)
                    nc.vector.tensor_mul(dst, dstg[:, 0:cols], dstg[:, 0:cols])

            def exit_copy(dst, src, cols):
                optA = {"A": cols * 0.833 + 396}
                optV = {"V": cols * 1.042 + 170}
                best = min((optA, optV), key=peak)
                add(best)
                if best is optA:
                    nc.scalar.copy(dst, src)
                else:
                    nc.vector.tensor_copy(dst, src)

            def mask_window(c2, w):
                """in-place triangular mask on a 128-col diagonal window;
                DVE 2X bf16 unless Pool is the lighter engine."""
                optV = {"V": 128 * 0.52 + 170}
                optP = {"P": 128 * 3.12 + 250}
                best = min((optV, optP), key=peak)
                add(best)
                eng = nc.vector if best is optV else nc.gpsimd
                eng.tensor_mul(c2[:, w:w + 128], c2[:, w:w + 128], trimask_sb[:])

            def emit_dpair(p, i, a):
                """D'[k, q] for key blocks (2a, 2a+1) vs query block i,
                narrowed to causal cols; returns (c2 tile, w0E, w0O)."""
                kcols = slice(a * 128, (a + 1) * 128)
                psAB = psd.tile([128, 2 * QB], F32, name="psAB", tag="psd")
                w0E = max(0, 128 * (2 * a) - QB * i)
                w0O = max(0, 128 * (2 * a + 1) - QB * i)
                nc.tensor.matmul(
                    psAB[:, w0E:QB], kT2[p][0:64, kcols],
                    qT2[p][0:64, i * QB + w0E:(i + 1) * QB],
                    start=True, stop=True, tile_position=(0, 0),
                    skip_group_check=True,
                )
                nc.tensor.matmul(
                    psAB[:, QB + w0O:2 * QB], kT2[p][64:128, kcols],
                    qT2[p][64:128, i * QB + w0O:(i + 1) * QB],
                    start=True, stop=True, tile_position=(64, 0),
                    skip_group_check=True,
                )
                c2 = cppool.tile([128, 2 * QB], BF16, name="c2", tag="cp")
                if w0E == 0 and w0O == 0:
                    # fused 2-bank exit amortizes the fixed per-op cost
                    exit_square(c2[:], psAB[:], 2 * QB)
                else:
                    exit_square(c2[:, w0E:QB], psAB[:, w0E:QB], QB - w0E)
                    exit_square(c2[:, QB + w0O:2 * QB], psAB[:, QB + w0O:2 * QB],
                                QB - w0O)
                for par, w0 in ((0, w0E), (1, w0O)):
                    r = 2 * a + par - (QB // 128) * i
                    if 0 <= r < QB // 128:
                        mask_window(c2, par * QB + w0)
                return c2, w0E, w0O

            def emit_cv(p, i, a, cur):
                c2, w0E, w0O = cur
                yzp = tails[(p, i)]
                vE = v1[p][:, (2 * a) * 65:(2 * a + 1) * 65]
                vO = v1[p][:, (2 * a + 1) * 65:(2 * a + 2) * 65]
                nc.tensor.matmul(
                    yzp[:, w0E:QB], vE, c2[:, w0E:QB],
                    start=(a == 0), stop=False, skip_group_check=True,
                )
                nc.tensor.matmul(
                    yzp[:, w0O:QB], vO, c2[:, QB + w0O:2 * QB],
                    start=False, stop=(a == 2 * i + 1), skip_group_check=True,
                )

            def emit_tail(p, i):
                yzs = smpool.tile([65, QB], BF16, name="yzs", tag="yzs")
                exit_copy(yzs[:], tails[(p, i)][:], QB)
                dmae = nc.sync if p == 0 else nc.scalar
                dmae.dma_start(outs[p][i * 65:(i + 1) * 65, :], yzs[:])

            AHEAD = 2  # super-steps between a dpair's D' and its CV
            seq = [(i, a) for i in (3, 2, 1, 0) for a in range(2 * i + 2)]
            dp = {}

            def emit_d(p, j):
                dp[(p, j)] = emit_dpair(p, *seq[j])

            def emit_c(p, j):
                i, a = seq[j]
                cur = dp.pop((p, j))
                if a == 0:
                    tails[(p, i)] = psyz.tile(
                        [65, QB], F32, name=f"yzp_{p}_{i}", tag="yzp"
                    )
                emit_cv(p, i, a, cur)
                if a == 2 * i + 1:
                    emit_tail(p, i)

            # D' units run AHEAD super-steps before their CV so the PE
            # always has an independent matmul between dependent ones and
            # the exit engines get a full pipeline of slack
            for j in range(len(seq) + AHEAD):
                for p in range(PAIRS):
                    if j < len(seq):
                        emit_d(p, j)
                for p in range(PAIRS):
                    if j >= AHEAD:
                        emit_c(p, j - AHEAD)

    nc.compile()
    return nc


def _shard_inputs(Q, K, V):
    """Per-core in_maps; core c gets global (b,h) pairs 2c and 2c+1.

    Host-side layout prep: bf16 cast + transpose into the exact SBUF
    layouts the device consumes (no device-side transposes/casts).
    """
    Q = np.asarray(Q, dtype=np.float32)
    K = np.asarray(K, dtype=np.float32)
    V = np.asarray(V, dtype=np.float32)
    b, t, h, d = Q.shape
    trimask = np.triu(np.ones((128, 128), dtype=np.float32)).astype(NP_BF16)
    in_maps = []
    for c in range(N_CORES):
        m = {"trimask": trimask}
        for p in range(PAIRS):
            g = PAIRS * c + p
            bb, hh = divmod(g, h)
            qT = np.ascontiguousarray(Q[bb, :, hh, :].T).astype(NP_BF16)  # [64, 2048]
            kT = np.ascontiguousarray(K[bb, :, hh, :].T).astype(NP_BF16)
            vp = V[bb, :, hh, :].astype(NP_BF16)                          # [2048, 64]
            m[f"qT2_{p}"] = np.concatenate([qT, qT], axis=0)              # [128, 2048]
            kT2 = np.empty((128, T // 2), dtype=NP_BF16)
            kTb = kT.reshape(64, NKB, 128)
            kT2[0:64] = kTb[:, 0::2, :].reshape(64, T // 2)
            kT2[64:128] = kTb[:, 1::2, :].reshape(64, T // 2)
            m[f"kT2_{p}"] = kT2
            v1 = np.ones((128, NKB, 65), dtype=NP_BF16)
            v1[:, :, 0:64] = vp.reshape(NKB, 128, 64).transpose(1, 0, 2)
            m[f"v1_{p}"] = v1.reshape(128, NKB * 65)
        in_maps.append(m)
    return in_maps


def kernel(Q, K, V, chunk_count, trace=False):
    Q = np.asarray(Q)
    b, t, h, d = Q.shape
    assert (b, t, h, d) == (2, T, 8, D), (b, t, h, d)
    assert T % int(chunk_count) == 0

    if "nc" not in _CACHE:
        _CACHE["nc"] = build_nc()
    nc = _CACHE["nc"]

    in_maps = _shard_inputs(Q, K, V)
    res = run_bass_kernel_spmd(nc, in_maps, core_ids=list(range(N_CORES)), trace=trace)

    out = np.empty((b, t, h, d), dtype=np.float32)
    for c in range(N_CORES):
        for p in range(PAIRS):
            g = PAIRS * c + p
            bb, hh = divmod(g, h)
            arr = np.asarray(res.results[c][f"o{p}"]).astype(np.float32)
            arr = arr.reshape(NQB, 65, QB)
            y = arr[:, :64, :]                      # [NQB, 64, QB]
            z = arr[:, 64, :]                       # [NQB, QB]
            outp = (y / z[:, None, :]).transpose(0, 2, 1).reshape(T, D)
            out[bb, :, hh, :] = outp
    if trace:
        return out, res
    return out
